# revision 20
# baseline (speedup 1.0000x reference)
"""Trainium2 Bass kernel: per-class precision/recall via a single mixed-dtype
fp8 gram pass.

Computes, for pred/gt 0-1 indicator tensors of shape [N, C]:
    intersection = sum_n pred*gt   [C]
    pred_sum     = sum_n pred      [C]
    gt_sum       = sum_n gt        [C]
    precisions   = (intersection + EPS) / (pred_sum + EPS)
    recalls      = (intersection + EPS) / (gt_sum + EPS)

Sharding: rows split across 8 NeuronCores. The host merges both indicators
into ONE byte per (row, class): v = p + 2g in {0,1,2,3}, shipped as the
e4m3 bytes {0x00, 0x38, 0x40, 0x44} -> values a = (0,1,2,3). Under the
e5m2 decoder the SAME bytes read b = (0, 0.5, 2, 4) -- a second,
independent nonlinear byte->value map, free in hardware (probe-verified:
a matmul with e4m3 weights and an e5m2-bitcast moving operand -- even one
aliasing the weight AP exactly -- computes the ordinary gram with the two
decoders applied per side).

With m_k = per-class #rows where v=k, ONE matmul per group recovers all
three needed functionals. Weights (lhsT, e4m3) = [v(126 cols) | ones | 0];
moving (rhs) = the SAME 128 columns BITCAST to e5m2:
    out[126, j] = sum_p 1 * b     = Sb_j     (ones WEIGHT column -> row)
    out[m, 126] = sum_p a * 0.5   = Sa_m / 2 (ones byte reads 0.5 in e5m2)
    out[j, j]   = sum_p a * b     = Sab_j    (gram diagonal)
    Sa  =  1 m1 + 2 m2 +  3 m3
    Sb  = .5 m1 + 2 m2 +  4 m3
    Sab = .5 m1 + 4 m2 + 12 m3
det = 3; the host solves in f64 (all sums are exact dyadics in fp32 PSUM)
and maps m -> (pred_sum = m1+m3, gt_sum = m2+m3, intersection = m3).

Column/class bookkeeping: each group carries 126 data columns, each column
= 128 rows of one (row-chunk, class). Groups are split into 8 CONTIGUOUS
blocks, one per psum bank (bank 0: 66 groups, banks 1-7: 65), so each
bank's accumulation finishes progressively through the run -- DVE copies
bank b out while PE is still matmul-ing later banks, and the output ships
as two DMAs that overlap the PE tail. Within bank b every group uses the
fixed column->class map (j + 2b) % 16; per-class capacity stays >= 4096
(the odd offsets make each class lose its 7-col shortfall in only one
bank). The host assigns row-chunks to columns sequentially per class and
pads the surplus with zero columns (harmless).

Device pipeline per core:
  - Input DMAs ride the two HWDGE queues (sync + scalar engines). The
    leading tiles are SMALL (8 groups first) so the first matmul starts
    ~4.5 us earlier than with uniform 1 MiB tiles; later tiles grow to
    full-bandwidth sizes.
  - TensorE: 521 matmuls (LDW 128 cols FWL + MM N=128, ~56 ns pace warm;
    ~67 ns when the chip is in the P0 power state), one per group.
  - DVE copies each psum bank as its block retires; outputs go out as two
    wide-row DMAs. (Narrow outputs are poison: a [128,1] f32 DMA =
    4-byte descriptor elements, measured ~5-8 us of DMA-engine slog.)

Measured on HW: 46.0 us best / ~51 us typical (run-to-run PE clock varies
2.0-2.4 GHz). Prior: two-tensor fp8 gram + DVE/PE gt-sum split: 65.6 us;
merged-byte + e5m2 N=1 bsums + DVE Sb reduces + wide outputs: 56.75 us.
"""

from contextlib import ExitStack

import numpy as np

N_CORES = 8
N_ROWS, C = 4194304, 16
ROWS_PER_CORE = N_ROWS // N_CORES  # 524288
EPS = np.float32(1e-6)

P = 128              # partitions = rows per column chunk
GCOLS = 128          # group: [v(126) | ones(1) | zero(1)]
DCOLS = 126          # data columns per group
N_BANKS = 8
BANK_SIZES = [66, 65, 65, 65, 65, 65, 65, 65]
BANK_OFF = np.cumsum([0] + BANK_SIZES).tolist()
N_GROUPS = BANK_OFF[-1]                       # 521
N_DATA_COLS = ROWS_PER_CORE // P * C          # 65536
N_CHUNKS = N_DATA_COLS // C                   # 4096 row-chunks per class
# leading tiles small (first-byte latency), trailing tiles big (bandwidth)
TILE_GROUPS = [8, 12, 20, 32, 48, 64, 88, 104, 145]
assert sum(TILE_GROUPS) == N_GROUPS
N_TILES = len(TILE_GROUPS)
TILE_OFF = np.cumsum([0] + TILE_GROUPS).tolist()

# v -> byte: e4m3 encodings of (0, 1, 2, 3); e5m2 decodes to (0, .5, 2, 4)
_V2BYTE = np.array([0x00, 0x38, 0x40, 0x44], np.uint8)
_F8_ONE = np.uint8(0x38)  # 1.0 in e4m3 (0.5 in e5m2)
# functional matrix rows: Sa, Sb, Sab over (m1, m2, m3); det = 3
_M = np.array([[1.0, 2.0, 3.0],
               [0.5, 2.0, 4.0],
               [0.5, 4.0, 12.0]])
_MINV = np.linalg.inv(_M)


def _col_maps():
    """Flat (bank-major) column index -> (class, chunk|-1 for pad)."""
    cls = np.concatenate([
        (np.arange(DCOLS)[None, :] + 2 * b).repeat(BANK_SIZES[b], 0).ravel()
        % 16
        for b in range(N_BANKS)
    ])
    order = np.argsort(cls, kind="stable")
    pos = np.empty(len(cls), np.int64)
    pos[order] = np.arange(len(cls))
    counts = np.bincount(cls, minlength=C)
    starts = np.concatenate([[0], np.cumsum(counts)[:-1]])
    rank = pos - starts[cls]
    chunk = np.where(rank < N_CHUNKS, rank, -1)
    return cls, chunk


_CLS, _CHUNK = _col_maps()

_CACHE = {}
LAST_RUN = None  # BassKernelResults of the most recent run (for test harness)


def _build_nc():
    import concourse.bass as bass
    import concourse.mybir as mybir

    f32 = mybir.dt.float32
    fp8 = mybir.dt.float8e4
    fp8e5 = mybir.dt.float8e5

    nc = bass.Bass()
    x_d = nc.dram_tensor("x", [P, N_GROUPS * GCOLS], fp8,
                         kind="ExternalInput")
    o_d = nc.dram_tensor("o", [P, N_BANKS * GCOLS], f32,
                         kind="ExternalOutput")

    ctx = ExitStack()
    with ctx:
        obuf = ctx.enter_context(
            nc.sbuf_tensor("obuf", [P, N_BANKS * GCOLS], f32))
        warm = ctx.enter_context(nc.sbuf_tensor("warm", [P, GCOLS], fp8))
        xbuf = ctx.enter_context(
            nc.sbuf_tensor("xbuf", [P, N_GROUPS * GCOLS], fp8))
        banks = [
            ctx.enter_context(nc.psum_tensor(f"pb{b}", [P, GCOLS], f32))
            for b in range(N_BANKS)
        ]

        tsems = [
            ctx.enter_context(nc.semaphore(name=f"t{t}"))
            for t in range(N_TILES)
        ]
        pe_sem = ctx.enter_context(nc.semaphore(name="pe"))
        v_sem = ctx.enter_context(nc.semaphore(name="vself"))
        out_sem = ctx.enter_context(nc.semaphore(name="outd"))
        block = ctx.enter_context(nc.Block(no_gpsimd_drain=True))

        def tile_slice(t):
            return TILE_OFF[t] * GCOLS, TILE_OFF[t + 1] * GCOLS

        @block.gpsimd
        def _(gpsimd):
            # SWDGE third stream: ~9 us of descriptor-generation ramp-up,
            # but issued at t~0 it hides under the prologue + early tiles;
            # it carries one big LATE tile, offloading the HWDGE queues
            lo, hi = tile_slice(N_TILES - 1)
            gpsimd.dma_start(xbuf[:, lo:hi],
                             x_d[:, lo:hi]).then_inc(tsems[N_TILES - 1], 16)

        @block.sync
        def _(sync):
            for t in range(0, N_TILES - 1, 2):
                lo, hi = tile_slice(t)
                sync.dma_start(xbuf[:, lo:hi],
                               x_d[:, lo:hi]).then_inc(tsems[t], 16)
            # outputs ship in three pieces overlapping the PE tail; the
            # final piece is a single bank so its completion is short
            c1, c2 = 4 * GCOLS, 7 * GCOLS
            sync.wait_ge(v_sem, 4)
            sync.dma_start(o_d[:, :c1], obuf[:, :c1]).then_inc(out_sem, 16)
            sync.wait_ge(v_sem, 7)
            sync.dma_start(o_d[:, c1:c2],
                           obuf[:, c1:c2]).then_inc(out_sem, 16)
            sync.wait_ge(v_sem, 8)
            sync.dma_start(o_d[:, c2:], obuf[:, c2:]).then_inc(out_sem, 16)
            sync.wait_ge(out_sem, 48)

        @block.scalar
        def _(scalar):
            for t in range(1, N_TILES - 1, 2):
                lo, hi = tile_slice(t)
                scalar.dma_start(xbuf[:, lo:hi],
                                 x_d[:, lo:hi]).then_inc(tsems[t], 16)

        @block.vector
        def _(vector):
            # bank b's accumulation ends with its contiguous block: copy
            # each bank as soon as its last matmul retires
            for b in range(N_BANKS):
                vector.wait_ge(pe_sem, b + 1)
                vector.tensor_scalar_mul(
                    obuf[:, b * GCOLS:(b + 1) * GCOLS], banks[b][:, :],
                    1.0).then_inc(v_sem, 1)

        @block.tensor
        def _(tensor):
            bank_of = np.searchsorted(BANK_OFF, np.arange(N_GROUPS),
                                      side="right") - 1
            # dummy matmuls while waiting for data: keeps the PE's HAM
            # activity monitor busy so the real matmuls start at 2.4 GHz
            # instead of the cold 1.2 GHz; bank 0's real chain opens with
            # start=True, which overwrites whatever these leave in PSUM
            for _ in range(64):
                nc.tensor.matmul(banks[0][:, :], warm[:, :], warm[:, :],
                                 start=True, stop=True)
            for t in range(N_TILES):
                tensor.wait_ge(tsems[t], 16)
                for g in range(TILE_OFF[t], TILE_OFF[t + 1]):
                    b = int(bank_of[g])
                    base = g * GCOLS
                    lhsT = xbuf[:, base:base + GCOLS]
                    inst = nc.tensor.matmul(
                        banks[b][:, :],
                        lhsT,
                        lhsT.bitcast(fp8e5),
                        start=(g == BANK_OFF[b]),
                        stop=(g == BANK_OFF[b + 1] - 1))
                    if g == BANK_OFF[b + 1] - 1:
                        inst.then_inc(pe_sem, 1)

    return nc


def _pack_core(pred_c, gt_c):
    """Stage one core's rows as [P, N_GROUPS*GCOLS] fp8 bytes (uint8)."""
    v = ((np.asarray(pred_c) != 0).astype(np.uint8)
         + 2 * (np.asarray(gt_c) != 0).astype(np.uint8))
    # chunk-columns: cols[k*16 + c] = class c's chunk k, [N_DATA_COLS, P]
    cols = _V2BYTE[v.reshape(N_CHUNKS, P, C)
                   .transpose(0, 2, 1).reshape(N_DATA_COLS, P)]
    X = np.zeros((N_GROUPS, GCOLS, P), np.uint8)
    data = X[:, :DCOLS, :]
    flat = np.zeros((N_GROUPS * DCOLS, P), np.uint8)
    live = _CHUNK >= 0
    flat[live] = cols[_CHUNK[live] * C + _CLS[live]]
    data[...] = flat.reshape(N_GROUPS, DCOLS, P)
    X[:, DCOLS, :] = _F8_ONE
    # -> [P, groups*cols]
    return X.reshape(N_GROUPS * GCOLS, P).T.copy()


def _unpack_out(o):
    """Fold one core's raw psum banks (f64) -> (Sa, Sb, Sab), each [C]."""
    o = o.reshape(P, N_BANKS, GCOLS).transpose(0, 2, 1)  # [m, n, b]
    j = np.arange(DCOLS)
    sa = np.zeros(C)
    sb = np.zeros(C)
    sab = np.zeros(C)
    for b in range(N_BANKS):
        cls = (j + 2 * b) % 16
        np.add.at(sa, cls, 2.0 * o[j, DCOLS, b])
        np.add.at(sb, cls, o[DCOLS, j, b])
        np.add.at(sab, cls, o[j, j, b])
    return sa, sb, sab


def _get_nc():
    if "nc" not in _CACHE:
        _CACHE["nc"] = _build_nc()
    return _CACHE["nc"]


def kernel(pred, gt, **run_kwargs):
    global LAST_RUN
    import ml_dtypes
    from concourse.bass_utils import run_bass_kernel_spmd

    pred = np.asarray(pred)
    gt = np.asarray(gt)
    assert pred.shape == (N_ROWS, C) and gt.shape == (N_ROWS, C)

    in_maps = []
    for i in range(N_CORES):
        sl = slice(i * ROWS_PER_CORE, (i + 1) * ROWS_PER_CORE)
        X = _pack_core(pred[sl], gt[sl])
        in_maps.append({"x": X.view(ml_dtypes.float8_e4m3)})

    nc = _get_nc()
    br = run_bass_kernel_spmd(nc, in_maps, core_ids=list(range(N_CORES)),
                              **run_kwargs)
    LAST_RUN = br

    sa = np.zeros(C)
    sb = np.zeros(C)
    sab = np.zeros(C)
    for r in br.results:
        a_, b_, ab_ = _unpack_out(r["o"].astype(np.float64))
        sa += a_
        sb += b_
        sab += ab_

    # solve [Sa, Sb, Sab] = M @ [m1, m2, m3] per class (exact integers)
    m = np.rint(_MINV @ np.stack([sa, sb, sab]))     # [3, C]
    inter = (m[2]).astype(np.float32)
    pred_sum = (m[0] + m[2]).astype(np.float32)
    gt_sum = (m[1] + m[2]).astype(np.float32)

    recalls = (inter + EPS) / (gt_sum + EPS)
    precisions = (inter + EPS) / (pred_sum + EPS)
    return (precisions, recalls, inter, gt_sum, pred_sum)
